# revision 9
# baseline (speedup 1.0000x reference)
"""Trainium2 Bass kernel for nn_DeltaNet_22488448762128 (v3).

Full-input contract: kernel(**inputs) takes unsharded numpy inputs, returns
the full [B, L, HID] output. Core = (b, hg): batch b in {0,1}, head-group hg
in {0..3} of HG=4 heads. Host sums the 4 partial outputs per batch + bo.

Design (per core, chunk c of 16 x 128 tokens):
  - PROJ: q/k/v = xT_blk.T @ W ; lhsT = x-block (f32r, self-loading matmul),
    rhs = W (bf16 moving, 1 cyc/row). PSUM q,k,v banks; drains on ACT; rope +
    phi on DVE (phi out bf16); khat = phik * (aC/a_t) in one DVE op.
  - beta/cumprod vectors (a, 1/a, aC/a, aC) computed on HOST from x@Wg and
    fed as small fp32 inputs (the g-projection is negligible host work).
  - q/k head-transposes (d-major for the scan) via ONE batched DMA-xbar
    transpose each (idle DMA engines; out AP [d, h, t], head-padded tile).
  - SCAN per head: A = kT.T@qT (bf16); A_sb = A*ainv[s]*mask; nu = A_sb.T@
    [V|1] + qT.T@S ; U = khat.T@[V|1]; S = aC*S + U; y = nu[:,:D]/(nu[:,D]+eps).
    nu|U|A packed in ONE psum bank (x2 bufs).
  - y batched DMA-transpose -> yT; OUTPROJ (2 chunks behind): out += yT.T@Wo
    (bf16 moving), 4x N=512 groups -> one [C, HID] store per chunk.
  - software pipeline per iter: PROJ(c), SCAN(c-1), OUTPROJ(c-2) so the PE
    never waits on DVE/DMA products; 16 warmup matmuls ramp the PE p-state.
"""

import math
import numpy as np

B, L, HID = 2, 2048, 2048
H, D = 16, 128
HG = 4
C = 128
NCHUNK = L // C
NK = HID // C
EPS = 1e-6
BETA_MIN, BETA_MAX = 0.8, 0.9995
NCORES = 8
GW = HG * D          # 512
half = D // 2
CP = C + 8           # padded head stride for 3D dma-transpose outputs

_CACHE = {}


def _rope_tables():
    inv_freq = (1.0 / (10000.0 ** (np.arange(half, dtype=np.float32) /
                                   np.float32(half)))).astype(np.float32)
    t = np.arange(L, dtype=np.float32)
    freqs = t[:, None] * inv_freq[None, :]
    cos = np.cos(freqs).astype(np.float32)
    sin = np.sin(freqs).astype(np.float32)

    def rearr(m):  # [L, half] -> [C, NCHUNK*half], block c at cols c*half
        return np.ascontiguousarray(
            m.reshape(NCHUNK, C, half).transpose(1, 0, 2).reshape(C, NCHUNK * half))
    return rearr(cos), rearr(sin)


def _build(cfg):
    import concourse.bass as bass
    import concourse.bacc as bacc
    import concourse.tile as tile
    import concourse.mybir as mybir
    from contextlib import ExitStack

    dt = mybir.dt
    F32 = dt.float32
    BF16 = dt.bfloat16
    F32R = dt.float32r
    Alu = mybir.AluOpType
    Act = mybir.ActivationFunctionType

    nch = cfg.get("nchunk", NCHUNK)
    nwarm = cfg.get("warm", 12)
    WMODE = cfg.get("wmode", "bf16")
    WDT = BF16 if WMODE == "bf16" else F32R
    XDT = BF16 if WMODE == "bf16" else F32R

    nc = bacc.Bacc("TRN2", target_bir_lowering=False, debug=False,
                   enable_asserts=False, num_devices=NCORES)

    # ---- DRAM I/O ----
    xT_d = nc.dram_tensor("xTb", [NCHUNK, C, HID], XDT, kind="ExternalInput").ap()
    wq_d = nc.dram_tensor("wq", [C, NK * GW], WDT, kind="ExternalInput").ap()
    wk_d = nc.dram_tensor("wk", [C, NK * GW], WDT, kind="ExternalInput").ap()
    wv_d = nc.dram_tensor("wv", [C, NK * GW], WDT, kind="ExternalInput").ap()
    wo_d = nc.dram_tensor("wo", [C, HG * HID], BF16, kind="ExternalInput").ap()
    cos_d = nc.dram_tensor("cosr", [C, NCHUNK * half], F32, kind="ExternalInput").ap()
    sin_d = nc.dram_tensor("sinr", [C, NCHUNK * half], F32, kind="ExternalInput").ap()
    mask_d = nc.dram_tensor("maskT", [C, C], F32, kind="ExternalInput").ap()
    idb_d = nc.dram_tensor("identb", [C, C], BF16, kind="ExternalInput").ap()
    ainv_d = nc.dram_tensor("ainv", [C, NCHUNK * HG], F32, kind="ExternalInput").ap()
    acdiv_d = nc.dram_tensor("acdiv", [C, NCHUNK * HG], F32, kind="ExternalInput").ap()
    acb_d = nc.dram_tensor("acb", [C, NCHUNK * HG], F32, kind="ExternalInput").ap()
    out_d = nc.dram_tensor("out", [L, HID], BF16, kind="ExternalOutput").ap()
    DBG = cfg.get("dbg", False)
    if DBG:
        dbg_q = nc.dram_tensor("dbg_q", [C, GW], F32, kind="ExternalOutput").ap()
        dbg_phiq = nc.dram_tensor("dbg_phiq", [C, GW], BF16, kind="ExternalOutput").ap()
        dbg_qT = nc.dram_tensor("dbg_qT", [C, HG * C], BF16, kind="ExternalOutput").ap()
        dbg_v = nc.dram_tensor("dbg_v", [C, HG * (D + 1)], BF16, kind="ExternalOutput").ap()
        dbg_y = nc.dram_tensor("dbg_y", [C, GW], BF16, kind="ExternalOutput").ap()
        dbg_w = nc.dram_tensor("dbg_w", [C, GW], WDT, kind="ExternalOutput").ap()
        dbg_kh = nc.dram_tensor("dbg_kh", [C, GW], BF16, kind="ExternalOutput").ap()

    NWSUB = 4               # weight quarters per projection
    KSUB = NK // NWSUB

    with ExitStack() as ctx:
        tc = ctx.enter_context(tile.TileContext(nc))

        cpool = ctx.enter_context(tc.tile_pool(name="consts", bufs=1))
        cos_t = cpool.tile([C, NCHUNK * half], F32, tag="cos")
        sin_t = cpool.tile([C, NCHUNK * half], F32, tag="sin")
        mask_t = cpool.tile([C, C], F32, tag="mask")
        ainv_t = cpool.tile([C, NCHUNK * HG], F32, tag="ainv")
        acdiv_t = cpool.tile([C, NCHUNK * HG], F32, tag="acdiv")
        acb_t = cpool.tile([C, NCHUNK * HG], F32, tag="acb")
        wsc = cpool.tile([C, GW + C], BF16, tag="wsc")   # warmup scratch
        idb_t = cpool.tile([C, C], BF16, tag="idb")

        wpool = ctx.enter_context(tc.tile_pool(name="w", bufs=1))
        wq_t = [wpool.tile([C, KSUB * GW], WDT, tag=f"wq{j}", name=f"wq{j}")
                for j in range(NWSUB)]
        wk_t = [wpool.tile([C, KSUB * GW], WDT, tag=f"wk{j}", name=f"wk{j}")
                for j in range(NWSUB)]
        wv_t = [wpool.tile([C, KSUB * GW], WDT, tag=f"wv{j}", name=f"wv{j}")
                for j in range(NWSUB)]
        wo_t = [wpool.tile([C, HID], BF16, tag=f"wo{h}", name=f"wo{h}")
                for h in range(HG)]

        # chunk-local SBUF pools
        xp = ctx.enter_context(tc.tile_pool(name="xp", bufs=cfg.get("xp", 2)))
        dr = ctx.enter_context(tc.tile_pool(name="dr", bufs=cfg.get("dr", 2)))
        rp = ctx.enter_context(tc.tile_pool(name="rp", bufs=2))
        ph = ctx.enter_context(tc.tile_pool(name="ph", bufs=2))
        tp = ctx.enter_context(tc.tile_pool(name="tp", bufs=2))
        sc = ctx.enter_context(tc.tile_pool(name="sc", bufs=3))
        spool = ctx.enter_context(tc.tile_pool(name="spool", bufs=2))
        ytb = ctx.enter_context(tc.tile_pool(name="ytb", bufs=2))
        ob = ctx.enter_context(tc.tile_pool(name="ob", bufs=cfg.get("ob", 2)))

        # PSUM: 3 (qkv) + 2 (scanps) + 2 (out) + 1 (warm) = 8 banks
        pqkv = ctx.enter_context(tc.tile_pool(name="pqkv", bufs=1, space="PSUM"))
        psc = ctx.enter_context(tc.tile_pool(name="psc", bufs=2, space="PSUM"))
        pou = ctx.enter_context(tc.tile_pool(name="pou", bufs=2, space="PSUM"))
        pwm = ctx.enter_context(tc.tile_pool(name="pwm", bufs=1, space="PSUM"))

        # ---- warmup the PE clock while initial DMAs land ----
        nc.vector.memset(wsc[:], 0.0)
        warm_ps = pwm.tile([C, GW], F32, tag="warm")
        tp_ps = pwm.tile([C, GW], BF16, tag="warm", name="tp_ps")
        for i in range(nwarm):
            nc.tensor.matmul(warm_ps[:], wsc[:, GW:GW + C], wsc[:, 0:GW],
                             start=True, stop=True, skip_group_check=True)

        # ---- initial DMAs: xtb(0) on SP; consts + weights on GPSIMD ----
        xtb_tiles = {}
        xtb_tiles[0] = xp.tile([C, HID], XDT, tag="xtb", name="xtb0")
        nc.sync.dma_start(xtb_tiles[0][:], xT_d[0])
        nc.scalar.dma_start(cos_t[:], cos_d)
        nc.scalar.dma_start(sin_t[:], sin_d)
        for j in range(NWSUB):
            js = bass.ts(j, KSUB * GW)
            nc.sync.dma_start(wq_t[j][:], wq_d[:, js])
            nc.scalar.dma_start(wv_t[j][:], wv_d[:, js])
            nc.sync.dma_start(wk_t[j][:], wk_d[:, js])
        nc.scalar.dma_start(acdiv_t[:], acdiv_d)
        nc.scalar.dma_start(ainv_t[:], ainv_d)
        nc.scalar.dma_start(acb_t[:], acb_d)
        nc.scalar.dma_start(mask_t[:], mask_d)
        nc.scalar.dma_start(idb_t[:], idb_d)
        for h in range(HG):
            nc.sync.dma_start(wo_t[h][:], wo_d[:, bass.ts(h, HID)])



        # ---- persistent state ----
        S_cur = []
        for h in range(HG):
            s0 = spool.tile([C, D + 1], BF16, tag=f"s{h}", name=f"s0_{h}")
            nc.vector.memset(s0[:], 0.0)
            S_cur.append(s0)

        stash = {}

        def emit_proj(c, tp_eng=None):
            xtb = xtb_tiles.pop(c)


            q_ps = pqkv.tile([C, GW], F32, tag="pq")
            k_ps = pqkv.tile([C, GW], F32, tag="pk")
            v_ps = pqkv.tile([C, GW], F32, tag="pv")
            for dst, wt in ((q_ps, wq_t), (k_ps, wk_t), (v_ps, wv_t)):
                for k in range(NK):
                    j, jj = divmod(k, KSUB)
                    nc.tensor.matmul(dst[:], xtb[:, bass.ts(k, C)],
                                     wt[j][:, bass.ts(jj, GW)],
                                     start=(k == 0), stop=(k == NK - 1))

            # drains (ACT)
            q_sb = dr.tile([C, GW], F32, tag="q")
            k_sb = dr.tile([C, GW], F32, tag="k")
            nc.scalar.copy(q_sb[:], q_ps[:])
            nc.scalar.copy(k_sb[:], k_ps[:])
            # rope (DVE)
            def rope(src, dst):
                se = src[:].rearrange("p (h d) -> p h d", h=HG)[:, :, 0:half]
                so = src[:].rearrange("p (h d) -> p h d", h=HG)[:, :, half:D]
                de = dst[:].rearrange("p (h d) -> p h d", h=HG)[:, :, 0:half]
                do = dst[:].rearrange("p (h d) -> p h d", h=HG)[:, :, half:D]
                cc = bass.AP(tensor=cos_t[:].tensor,
                             offset=cos_t[:, bass.ts(c, half)].offset,
                             ap=[cos_t[:].ap[0], [0, HG], [1, half]])
                ss = bass.AP(tensor=sin_t[:].tensor,
                             offset=sin_t[:, bass.ts(c, half)].offset,
                             ap=[sin_t[:].ap[0], [0, HG], [1, half]])
                tmp = rp.tile([C, GW], F32, tag="rtmp")
                t1 = tmp[:].rearrange("p (h d) -> p h d", h=HG)[:, :, 0:half]
                t2 = tmp[:].rearrange("p (h d) -> p h d", h=HG)[:, :, half:D]
                nc.vector.tensor_tensor(out=t1, in0=se, in1=cc, op=Alu.mult)
                nc.vector.tensor_tensor(out=t2, in0=so, in1=ss, op=Alu.mult)
                nc.vector.tensor_tensor(out=de, in0=t1, in1=t2, op=Alu.subtract)
                nc.vector.tensor_tensor(out=t1, in0=se, in1=ss, op=Alu.mult)
                nc.vector.tensor_tensor(out=t2, in0=so, in1=cc, op=Alu.mult)
                nc.vector.tensor_tensor(out=do, in0=t1, in1=t2, op=Alu.add)

            qr = rp.tile([C, GW], F32, tag="qr")
            kr = rp.tile([C, GW], F32, tag="kr")
            rope(q_sb, qr)
            rope(k_sb, kr)

            # phi = exp(min(x,0)) + relu(x)  (gpsimd min, ACT exp, DVE fuse)
            def phi(src, nm):
                tm = rp.tile([C, GW], F32, tag="rtmp")
                nc.vector.tensor_scalar(out=tm[:], in0=src[:], scalar1=0.0,
                                        scalar2=None, op0=Alu.min)
                te = rp.tile([C, GW], F32, tag="te")
                nc.scalar.activation(te[:], tm[:], Act.Exp)
                p = ph.tile([C, GW], BF16, tag=nm)
                nc.vector.scalar_tensor_tensor(out=p[:], in0=src[:],
                                               scalar=0.0, in1=te[:],
                                               op0=Alu.max, op1=Alu.add)
                return p

            phiq = phi(qr, "phiq")
            phik = phi(kr, "phik")

            # khat for all 4 heads in one DVE op: phik * acdiv[t,h] (bcast on d)
            khat = ph.tile([C, GW], BF16, tag="khat")
            av = bass.AP(tensor=acdiv_t[:].tensor,
                         offset=acdiv_t[:, bass.ts(c, HG)].offset,
                         ap=[acdiv_t[:].ap[0], [1, HG], [0, D]])
            nc.vector.tensor_tensor(
                out=khat[:].rearrange("p (h d) -> p h d", h=HG),
                in0=phik[:].rearrange("p (h d) -> p h d", h=HG),
                in1=av, op=Alu.mult)

            # per-head PE transposes -> [d, t] tiles (v1-proven path).
            # two regions x (q,k): head h uses region h%2 so head h+1's
            # transposes overlap head h's ACT drains.
            qT3 = tp.tile([C, HG, C], BF16, tag="qT3")
            kT3 = tp.tile([C, HG, C], BF16, tag="kT3")
            for h in range(HG):
                r = h % 2
                nc.tensor.transpose(tp_ps[:, r * C:(r + 1) * C],
                                    phiq[:, bass.ts(h, D)], idb_t[:])
                nc.tensor.transpose(tp_ps[:, (2 + r) * C:(3 + r) * C],
                                    phik[:, bass.ts(h, D)], idb_t[:])
                nc.scalar.copy(qT3[:, h, :], tp_ps[:, r * C:(r + 1) * C])
                nc.scalar.copy(kT3[:, h, :], tp_ps[:, (2 + r) * C:(3 + r) * C])
            if c + 1 < nch:
                nxt = xp.tile([C, HID], XDT, tag="xtb", name=f"xtb{c + 1}")
                nc.sync.dma_start(nxt[:], xT_d[c + 1])
                xtb_tiles[c + 1] = nxt

            # v drain late (keeps exp/phi ahead of it on ACT; needed next iter)
            v_sb = ph.tile([C, HG * (D + 1)], BF16, tag="v")
            v_aug = v_sb[:].rearrange("p (h e) -> p h e", e=D + 1)
            nc.scalar.copy(v_aug[:, :, 0:D],
                           v_ps[:].rearrange("p (h e) -> p h e", e=D))
            nc.vector.memset(v_aug[:, :, D:D + 1], 1.0)

            if DBG and c == 0:
                nc.sync.dma_start(dbg_q, q_sb[:])
                nc.sync.dma_start(dbg_phiq, phiq[:])
                nc.sync.dma_start(dbg_qT, qT3[:].rearrange("p h t -> p (h t)"))
                nc.sync.dma_start(dbg_v, v_sb[:])
                nc.sync.dma_start(dbg_w, wq_t[0][:, 0:GW])
                nc.sync.dma_start(dbg_kh, khat[:])
            stash[c] = dict(qT3=qT3, kT3=kT3, khat=khat, v_sb=v_sb)

        def emit_scan(c, inter=None, ytp_eng=None):
            st = stash.pop(c)
            qT3, kT3, khat, v_sb = st["qT3"], st["kT3"], st["khat"], st["v_sb"]
            y_all = sc.tile([C, GW], BF16, tag="yall", bufs=2)
            for h in range(HG):
                if inter is not None:
                    inter(h)
                ps = psc.tile([C, 2 * (D + 1) + C], F32, tag="scanps")
                nu = ps[:, 0:D + 1]
                U = ps[:, D + 1:2 * (D + 1)]
                A_ps = ps[:, 2 * (D + 1):2 * (D + 1) + C]
                vh = v_sb[:, h * (D + 1):(h + 1) * (D + 1)]
                qT = qT3[:, h, :]
                kT = kT3[:, h, :]

                nc.tensor.matmul(A_ps, kT, qT, start=True, stop=True)
                A_sb = sc.tile([C, C], BF16, tag="A")
                nc.vector.scalar_tensor_tensor(
                    out=A_sb[:], in0=A_ps,
                    scalar=ainv_t[:, c * HG + h:c * HG + h + 1], in1=mask_t[:],
                    op0=Alu.mult, op1=Alu.mult)

                nc.tensor.matmul(U, khat[:, bass.ts(h, D)], vh,
                                 start=True, stop=True)
                nc.tensor.matmul(nu, A_sb[:], vh, start=True, stop=False)
                nc.tensor.matmul(nu, qT, S_cur[h][:], start=False, stop=True)

                S_new = spool.tile([C, D + 1], BF16, tag=f"s{h}")
                nc.vector.scalar_tensor_tensor(
                    out=S_new[:], in0=S_cur[h][:],
                    scalar=acb_t[:, c * HG + h:c * HG + h + 1],
                    in1=U, op0=Alu.mult, op1=Alu.add)
                S_cur[h] = S_new

                rd = sc.tile([C, 1], F32, tag="rd")
                nc.vector.tensor_scalar_add(rd[:], nu[:, D:D + 1], EPS)
                nc.vector.reciprocal(rd[:], rd[:])
                nc.vector.tensor_scalar_mul(y_all[:, bass.ts(h, D)],
                                            nu[:, 0:D], rd[:])

            if DBG and c == 0:
                nc.sync.dma_start(dbg_y, y_all[:])
            yT3 = ytb.tile([C, HG, C], BF16, tag="yT3")
            for h in range(HG):
                r = h % 2
                nc.tensor.transpose(tp_ps[:, r * C:(r + 1) * C],
                                    y_all[:, bass.ts(h, D)], idb_t[:])
                if h % 2 == 0:
                    nc.scalar.copy(yT3[:, h, :], tp_ps[:, r * C:(r + 1) * C])
                else:
                    nc.vector.tensor_copy(yT3[:, h, :],
                                          tp_ps[:, r * C:(r + 1) * C])
            stash[(c, "yT3")] = yT3

        def outproj_group(c, o, yT3, out_sb, split_store):
            out_ps = pou.tile([C, GW], F32, tag="po")
            for h in range(HG):
                nc.tensor.matmul(out_ps[:], yT3[:, h, :],
                                 wo_t[h][:, bass.ts(o, GW)],
                                 start=(h == 0), stop=(h == HG - 1))
            nc.scalar.copy(out_sb[:, bass.ts(o, GW)], out_ps[:])
            if split_store:
                nc.sync.dma_start(out_d[bass.ts(c, C), bass.ts(o, GW)],
                                  out_sb[:, bass.ts(o, GW)])

        def emit_outproj(c, split_store=False, group_cb=None):
            yT3 = stash.pop((c, "yT3"))
            out_sb = ob.tile([C, HID], BF16, tag="osb")
            NO = HID // GW
            if group_cb is not None:
                return yT3, out_sb
            for o in range(NO):
                outproj_group(c, o, yT3, out_sb, split_store)
            if not split_store:
                nc.sync.dma_start(out_d[bass.ts(c, C), :], out_sb[:])

        for c in range(nch):
            emit_proj(c, tp_eng=nc.scalar if c == nch - 1 else None)
            if c >= 1:
                emit_scan(c - 1)
            if c >= 2:
                emit_outproj(c - 2)
        if nch >= 2:
            yT3_p, osb_p = emit_outproj(nch - 2, split_store=True, group_cb=True)
            emit_scan(nch - 1,
                      inter=lambda h: outproj_group(nch - 2, h, yT3_p, osb_p,
                                                    True),
                      ytp_eng=None)
        else:
            emit_scan(nch - 1)
        emit_outproj(nch - 1, split_store=True)

    nc.compile()
    return nc


def _get_nc(cfg_key="default", **cfg):
    if cfg_key not in _CACHE:
        _CACHE[cfg_key] = _build(cfg)
    return _CACHE[cfg_key]


def _bf16(a):
    import ml_dtypes
    return np.ascontiguousarray(a).astype(ml_dtypes.bfloat16)


def _block_w(W, fw):
    # [HID, fw] -> [C, NK*fw] with block k = W[k*128:(k+1)*128, :]
    return np.ascontiguousarray(
        W.reshape(NK, C, fw).transpose(1, 0, 2).reshape(C, NK * fw))


def make_in_maps(x, Wq, Wk, Wv, Wg, bg, Wo, bo, wdt="bf16"):
    cosr, sinr = _rope_tables()
    maskT = np.triu(np.ones((C, C), np.float32))
    import ml_dtypes
    identb = np.eye(C, dtype=ml_dtypes.bfloat16)
    x = np.asarray(x, np.float32)
    Wq, Wk, Wv = np.asarray(Wq), np.asarray(Wk), np.asarray(Wv)
    Wg, bg, Wo = np.asarray(Wg), np.asarray(bg), np.asarray(Wo)
    wcast = _bf16 if wdt == "bf16" else (
        lambda a: np.ascontiguousarray(a).astype(np.float32))
    xcast = _bf16 if wdt == "bf16" else (
        lambda a: np.ascontiguousarray(a).astype(np.float32))

    # host beta pipeline: beta = clip(sigmoid(x@Wg+bg)); per-chunk cumprods
    g = x.reshape(B * L, HID) @ Wg + bg          # [B*L, H]
    beta = np.clip(1.0 / (1.0 + np.exp(-g)), BETA_MIN, BETA_MAX)
    beta = beta.reshape(B, L, H).astype(np.float32)
    a = np.cumprod(beta.reshape(B, NCHUNK, C, H), axis=2).astype(np.float32)
    aC = a[:, :, C - 1:C, :]                      # [B, NCHUNK, 1, H]
    ainv = (1.0 / a).astype(np.float32)
    acdiv = (aC / a).astype(np.float32)
    acb = np.broadcast_to(aC, a.shape).astype(np.float32)

    def bvec(m, b, hsl):  # [B,NCHUNK,C,H] -> [C, NCHUNK*HG]
        return np.ascontiguousarray(
            m[b][:, :, hsl].transpose(1, 0, 2).reshape(C, NCHUNK * HG)
        ).astype(np.float32)

    in_maps = []
    for core in range(NCORES):
        b, hg = divmod(core, 4)
        cs = slice(hg * GW, (hg + 1) * GW)
        hsl = slice(hg * HG, (hg + 1) * HG)
        xTb = xcast(
            x[b].reshape(NCHUNK, C, NK, C).transpose(0, 3, 2, 1)
            .reshape(NCHUNK, C, HID))
        in_maps.append({
            "xTb": xTb,
            "wq": wcast(_block_w(Wq[:, cs], GW)),
            "wk": wcast(_block_w(Wk[:, cs], GW)),
            "wv": wcast(_block_w(Wv[:, cs], GW)),
            "wo": _bf16(Wo[cs, :].reshape(HG, C, HID).transpose(1, 0, 2)
                        .reshape(C, HG * HID)),
            "cosr": cosr, "sinr": sinr, "maskT": maskT,
            "identb": identb,
            "ainv": bvec(ainv, b, hsl),
            "acdiv": bvec(acdiv, b, hsl),
            "acb": bvec(acb, b, hsl),
        })
    return in_maps


def kernel(x, Wq, Wk, Wv, Wg, bg, Wo, bo, _trace=False, cfg_key="default", **cfg):
    from concourse.bass_utils import run_bass_kernel_spmd
    nc = _get_nc(cfg_key=cfg_key, **cfg)
    in_maps = make_in_maps(x, Wq, Wk, Wv, Wg, bg, Wo, bo,
                           wdt=cfg.get("wmode", "bf16"))
    res = run_bass_kernel_spmd(nc, in_maps, core_ids=list(range(NCORES)),
                               trace=_trace)
    out = np.zeros((B, L, HID), np.float32)
    for core in range(NCORES):
        b = core // 4
        out[b] += np.asarray(res.results[core]["out"], np.float32)
    out += np.asarray(bo, np.float32)[None, None, :]
    kernel._last_results = res
    return out


# revision 10
# speedup vs baseline: 1.0659x; 1.0659x over previous
"""Trainium2 Bass kernel for nn_DeltaNet_22488448762128 (v3).

Full-input contract: kernel(**inputs) takes unsharded numpy inputs, returns
the full [B, L, HID] output. Core = (b, hg): batch b in {0,1}, head-group hg
in {0..3} of HG=4 heads. Host sums the 4 partial outputs per batch + bo.

Design (per core, chunk c of 16 x 128 tokens):
  - PROJ: q/k/v = xT_blk.T @ W ; lhsT = x-block (f32r, self-loading matmul),
    rhs = W (bf16 moving, 1 cyc/row). PSUM q,k,v banks; drains on ACT; rope +
    phi on DVE (phi out bf16); khat = phik * (aC/a_t) in one DVE op.
  - beta/cumprod vectors (a, 1/a, aC/a, aC) computed on HOST from x@Wg and
    fed as small fp32 inputs (the g-projection is negligible host work).
  - q/k head-transposes (d-major for the scan) via ONE batched DMA-xbar
    transpose each (idle DMA engines; out AP [d, h, t], head-padded tile).
  - SCAN per head: A = kT.T@qT (bf16); A_sb = A*ainv[s]*mask; nu = A_sb.T@
    [V|1] + qT.T@S ; U = khat.T@[V|1]; S = aC*S + U; y = nu[:,:D]/(nu[:,D]+eps).
    nu|U|A packed in ONE psum bank (x2 bufs).
  - y batched DMA-transpose -> yT; OUTPROJ (2 chunks behind): out += yT.T@Wo
    (bf16 moving), 4x N=512 groups -> one [C, HID] store per chunk.
  - software pipeline per iter: PROJ(c), SCAN(c-1), OUTPROJ(c-2) so the PE
    never waits on DVE/DMA products; 16 warmup matmuls ramp the PE p-state.
"""

import math
import numpy as np

B, L, HID = 2, 2048, 2048
H, D = 16, 128
HG = 4
C = 128
NCHUNK = L // C
NK = HID // C
EPS = 1e-6
BETA_MIN, BETA_MAX = 0.8, 0.9995
NCORES = 8
GW = HG * D          # 512
half = D // 2
CP = C + 8           # padded head stride for 3D dma-transpose outputs

_CACHE = {}


def _rope_tables():
    inv_freq = (1.0 / (10000.0 ** (np.arange(half, dtype=np.float32) /
                                   np.float32(half)))).astype(np.float32)
    t = np.arange(L, dtype=np.float32)
    freqs = t[:, None] * inv_freq[None, :]
    cos = np.cos(freqs).astype(np.float32)
    sin = np.sin(freqs).astype(np.float32)

    def rearr(m):  # [L, half] -> [C, NCHUNK*half], block c at cols c*half
        return np.ascontiguousarray(
            m.reshape(NCHUNK, C, half).transpose(1, 0, 2).reshape(C, NCHUNK * half))
    return rearr(cos), rearr(sin)


def _build(cfg):
    import concourse.bass as bass
    import concourse.bacc as bacc
    import concourse.tile as tile
    import concourse.mybir as mybir
    from contextlib import ExitStack

    dt = mybir.dt
    F32 = dt.float32
    BF16 = dt.bfloat16
    F32R = dt.float32r
    Alu = mybir.AluOpType
    Act = mybir.ActivationFunctionType

    nch = cfg.get("nchunk", NCHUNK)
    nwarm = cfg.get("warm", 12)
    WMODE = cfg.get("wmode", "bf16")
    WDT = BF16 if WMODE == "bf16" else F32R
    XDT = BF16 if WMODE == "bf16" else F32R

    nc = bacc.Bacc("TRN2", target_bir_lowering=False, debug=False,
                   enable_asserts=False, num_devices=NCORES)

    # ---- DRAM I/O ----
    xT_d = nc.dram_tensor("xTb", [NCHUNK, C, HID], XDT, kind="ExternalInput").ap()
    wq_d = nc.dram_tensor("wq", [C, NK * GW], WDT, kind="ExternalInput").ap()
    wk_d = nc.dram_tensor("wk", [C, NK * GW], WDT, kind="ExternalInput").ap()
    wv_d = nc.dram_tensor("wv", [C, NK * GW], WDT, kind="ExternalInput").ap()
    wo_d = nc.dram_tensor("wo", [C, HG * HID], BF16, kind="ExternalInput").ap()
    cos_d = nc.dram_tensor("cosr", [C, NCHUNK * half], F32, kind="ExternalInput").ap()
    sin_d = nc.dram_tensor("sinr", [C, NCHUNK * half], F32, kind="ExternalInput").ap()
    mask_d = nc.dram_tensor("maskT", [C, C], F32, kind="ExternalInput").ap()
    idb_d = nc.dram_tensor("identb", [C, C], BF16, kind="ExternalInput").ap()
    ainv_d = nc.dram_tensor("ainv", [C, NCHUNK * HG], F32, kind="ExternalInput").ap()
    acdiv_d = nc.dram_tensor("acdiv", [C, NCHUNK * HG], F32, kind="ExternalInput").ap()
    acb_d = nc.dram_tensor("acb", [C, NCHUNK * HG], F32, kind="ExternalInput").ap()
    out_d = nc.dram_tensor("out", [L, HID], BF16, kind="ExternalOutput").ap()
    DBG = cfg.get("dbg", False)
    if DBG:
        dbg_q = nc.dram_tensor("dbg_q", [C, GW], F32, kind="ExternalOutput").ap()
        dbg_phiq = nc.dram_tensor("dbg_phiq", [C, GW], BF16, kind="ExternalOutput").ap()
        dbg_qT = nc.dram_tensor("dbg_qT", [C, HG * C], BF16, kind="ExternalOutput").ap()
        dbg_v = nc.dram_tensor("dbg_v", [C, HG * (D + 1)], BF16, kind="ExternalOutput").ap()
        dbg_y = nc.dram_tensor("dbg_y", [C, GW], BF16, kind="ExternalOutput").ap()
        dbg_w = nc.dram_tensor("dbg_w", [C, GW], WDT, kind="ExternalOutput").ap()
        dbg_kh = nc.dram_tensor("dbg_kh", [C, GW], BF16, kind="ExternalOutput").ap()

    NWSUB = 4               # weight quarters per projection
    KSUB = NK // NWSUB

    with ExitStack() as ctx:
        tc = ctx.enter_context(tile.TileContext(nc))

        cpool = ctx.enter_context(tc.tile_pool(name="consts", bufs=1))
        cos_t = cpool.tile([C, NCHUNK * half], F32, tag="cos")
        sin_t = cpool.tile([C, NCHUNK * half], F32, tag="sin")
        mask_t = cpool.tile([C, C], F32, tag="mask")
        ainv_t = cpool.tile([C, NCHUNK * HG], F32, tag="ainv")
        acdiv_t = cpool.tile([C, NCHUNK * HG], F32, tag="acdiv")
        acb_t = cpool.tile([C, NCHUNK * HG], F32, tag="acb")
        wsc = cpool.tile([C, GW + C], BF16, tag="wsc")   # warmup scratch
        idb_t = cpool.tile([C, C], BF16, tag="idb")

        wpool = ctx.enter_context(tc.tile_pool(name="w", bufs=1))
        wq_t = [wpool.tile([C, KSUB * GW], WDT, tag=f"wq{j}", name=f"wq{j}")
                for j in range(NWSUB)]
        wk_t = [wpool.tile([C, KSUB * GW], WDT, tag=f"wk{j}", name=f"wk{j}")
                for j in range(NWSUB)]
        wv_t = [wpool.tile([C, KSUB * GW], WDT, tag=f"wv{j}", name=f"wv{j}")
                for j in range(NWSUB)]
        wo_t = [wpool.tile([C, HID], BF16, tag=f"wo{h}", name=f"wo{h}")
                for h in range(HG)]

        # chunk-local SBUF pools
        xp = ctx.enter_context(tc.tile_pool(name="xp", bufs=cfg.get("xp", 2)))
        dr = ctx.enter_context(tc.tile_pool(name="dr", bufs=cfg.get("dr", 2)))
        rp = ctx.enter_context(tc.tile_pool(name="rp", bufs=2))
        ph = ctx.enter_context(tc.tile_pool(name="ph", bufs=2))
        tp = ctx.enter_context(tc.tile_pool(name="tp", bufs=2))
        sc = ctx.enter_context(tc.tile_pool(name="sc", bufs=3))
        spool = ctx.enter_context(tc.tile_pool(name="spool", bufs=2))
        ytb = ctx.enter_context(tc.tile_pool(name="ytb", bufs=2))
        ob = ctx.enter_context(tc.tile_pool(name="ob", bufs=cfg.get("ob", 2)))

        # PSUM: 3 (qkv) + 2 (scanps) + 2 (out) + 1 (warm) = 8 banks
        pqkv = ctx.enter_context(tc.tile_pool(name="pqkv", bufs=1, space="PSUM"))
        psc = ctx.enter_context(tc.tile_pool(name="psc", bufs=2, space="PSUM"))
        pou = ctx.enter_context(tc.tile_pool(name="pou", bufs=2, space="PSUM"))
        pwm = ctx.enter_context(tc.tile_pool(name="pwm", bufs=1, space="PSUM"))

        # ---- warmup the PE clock while initial DMAs land ----
        nc.vector.memset(wsc[:], 0.0)
        warm_ps = pwm.tile([C, GW], F32, tag="warm")
        for i in range(nwarm):
            nc.tensor.matmul(warm_ps[:], wsc[:, GW:GW + C], wsc[:, 0:GW],
                             start=True, stop=True, skip_group_check=True)

        # ---- initial DMAs: xtb(0) on SP; consts + weights on GPSIMD ----
        xtb_tiles = {}
        xtb_tiles[0] = xp.tile([C, HID], XDT, tag="xtb", name="xtb0")
        nc.sync.dma_start(xtb_tiles[0][:], xT_d[0])
        nc.scalar.dma_start(cos_t[:], cos_d)
        nc.scalar.dma_start(sin_t[:], sin_d)
        for wd, wt in ((wq_d, wq_t), (wk_d, wk_t), (wv_d, wv_t)):
            for j in range(NWSUB):
                js = bass.ts(j, KSUB * GW)
                nc.gpsimd.dma_start(wt[j][:], wd[:, js])
        nc.scalar.dma_start(acdiv_t[:], acdiv_d)
        nc.scalar.dma_start(ainv_t[:], ainv_d)
        nc.scalar.dma_start(acb_t[:], acb_d)
        nc.scalar.dma_start(mask_t[:], mask_d)
        nc.scalar.dma_start(idb_t[:], idb_d)
        for h in range(HG):
            for u in range(2):
                nc.gpsimd.dma_start(
                    wo_t[h][:, bass.ts(u, HID // 2)],
                    wo_d[:, h * HID + u * (HID // 2):
                         h * HID + (u + 1) * (HID // 2)])



        # ---- persistent state ----
        S_cur = []
        for h in range(HG):
            s0 = spool.tile([C, D + 1], BF16, tag=f"s{h}", name=f"s0_{h}")
            nc.vector.memset(s0[:], 0.0)
            S_cur.append(s0)

        stash = {}

        def emit_proj(c, tp_eng=None):
            xtb = xtb_tiles.pop(c)


            q_ps = pqkv.tile([C, GW], F32, tag="pq")
            k_ps = pqkv.tile([C, GW], F32, tag="pk")
            v_ps = pqkv.tile([C, GW], F32, tag="pv")
            for dst, wt in ((q_ps, wq_t), (k_ps, wk_t), (v_ps, wv_t)):
                for k in range(NK):
                    j, jj = divmod(k, KSUB)
                    nc.tensor.matmul(dst[:], xtb[:, bass.ts(k, C)],
                                     wt[j][:, bass.ts(jj, GW)],
                                     start=(k == 0), stop=(k == NK - 1))

            # drains (ACT)
            q_sb = dr.tile([C, GW], F32, tag="q")
            k_sb = dr.tile([C, GW], F32, tag="k")
            nc.scalar.copy(q_sb[:], q_ps[:])
            nc.scalar.copy(k_sb[:], k_ps[:])
            # rope (DVE)
            def rope(src, dst):
                se = src[:].rearrange("p (h d) -> p h d", h=HG)[:, :, 0:half]
                so = src[:].rearrange("p (h d) -> p h d", h=HG)[:, :, half:D]
                de = dst[:].rearrange("p (h d) -> p h d", h=HG)[:, :, 0:half]
                do = dst[:].rearrange("p (h d) -> p h d", h=HG)[:, :, half:D]
                cc = bass.AP(tensor=cos_t[:].tensor,
                             offset=cos_t[:, bass.ts(c, half)].offset,
                             ap=[cos_t[:].ap[0], [0, HG], [1, half]])
                ss = bass.AP(tensor=sin_t[:].tensor,
                             offset=sin_t[:, bass.ts(c, half)].offset,
                             ap=[sin_t[:].ap[0], [0, HG], [1, half]])
                tmp = rp.tile([C, GW], F32, tag="rtmp")
                t1 = tmp[:].rearrange("p (h d) -> p h d", h=HG)[:, :, 0:half]
                t2 = tmp[:].rearrange("p (h d) -> p h d", h=HG)[:, :, half:D]
                nc.vector.tensor_tensor(out=t1, in0=se, in1=cc, op=Alu.mult)
                nc.vector.tensor_tensor(out=t2, in0=so, in1=ss, op=Alu.mult)
                nc.vector.tensor_tensor(out=de, in0=t1, in1=t2, op=Alu.subtract)
                nc.vector.tensor_tensor(out=t1, in0=se, in1=ss, op=Alu.mult)
                nc.vector.tensor_tensor(out=t2, in0=so, in1=cc, op=Alu.mult)
                nc.vector.tensor_tensor(out=do, in0=t1, in1=t2, op=Alu.add)

            qr = rp.tile([C, GW], F32, tag="qr")
            kr = rp.tile([C, GW], F32, tag="kr")
            rope(q_sb, qr)
            rope(k_sb, kr)

            # phi = exp(min(x,0)) + relu(x)  (gpsimd min, ACT exp, DVE fuse)
            def phi(src, nm):
                tm = rp.tile([C, GW], F32, tag="rtmp")
                nc.vector.tensor_scalar(out=tm[:], in0=src[:], scalar1=0.0,
                                        scalar2=None, op0=Alu.min)
                te = rp.tile([C, GW], F32, tag="te")
                nc.scalar.activation(te[:], tm[:], Act.Exp)
                p = ph.tile([C, GW], BF16, tag=nm)
                nc.vector.scalar_tensor_tensor(out=p[:], in0=src[:],
                                               scalar=0.0, in1=te[:],
                                               op0=Alu.max, op1=Alu.add)
                return p

            phiq = phi(qr, "phiq")
            phik = phi(kr, "phik")

            # khat for all 4 heads in one DVE op: phik * acdiv[t,h] (bcast on d)
            khat = ph.tile([C, GW], BF16, tag="khat")
            av = bass.AP(tensor=acdiv_t[:].tensor,
                         offset=acdiv_t[:, bass.ts(c, HG)].offset,
                         ap=[acdiv_t[:].ap[0], [1, HG], [0, D]])
            nc.vector.tensor_tensor(
                out=khat[:].rearrange("p (h d) -> p h d", h=HG),
                in0=phik[:].rearrange("p (h d) -> p h d", h=HG),
                in1=av, op=Alu.mult)

            # batched DMA xbar transposes: [t, (h d)] -> [d, h, t]
            # (contiguous out tile; HW-validated in micro1)
            qT3 = tp.tile([C, HG, C], BF16, tag="qT3")
            kT3 = tp.tile([C, HG, C], BF16, tag="kT3")
            (tp_eng or nc.sync).dma_start(qT3[:], phiq[:], transpose=True)
            (tp_eng or nc.sync).dma_start(kT3[:], phik[:], transpose=True)
            if c + 1 < nch:
                nxt = xp.tile([C, HID], XDT, tag="xtb", name=f"xtb{c + 1}")
                nc.sync.dma_start(nxt[:], xT_d[c + 1])
                xtb_tiles[c + 1] = nxt

            # v drain late (keeps exp/phi ahead of it on ACT; needed next iter)
            v_sb = ph.tile([C, HG * (D + 1)], BF16, tag="v")
            v_aug = v_sb[:].rearrange("p (h e) -> p h e", e=D + 1)
            nc.scalar.copy(v_aug[:, :, 0:D],
                           v_ps[:].rearrange("p (h e) -> p h e", e=D))
            nc.vector.memset(v_aug[:, :, D:D + 1], 1.0)

            if DBG and c == 0:
                nc.sync.dma_start(dbg_q, q_sb[:])
                nc.sync.dma_start(dbg_phiq, phiq[:])
                nc.sync.dma_start(dbg_qT, qT3[:].rearrange("p h t -> p (h t)"))
                nc.sync.dma_start(dbg_v, v_sb[:])
                nc.sync.dma_start(dbg_w, wq_t[0][:, 0:GW])
                nc.sync.dma_start(dbg_kh, khat[:])
            stash[c] = dict(qT3=qT3, kT3=kT3, khat=khat, v_sb=v_sb)

        def emit_scan(c, inter=None, ytp_eng=None):
            st = stash.pop(c)
            qT3, kT3, khat, v_sb = st["qT3"], st["kT3"], st["khat"], st["v_sb"]
            y_all = sc.tile([C, GW], BF16, tag="yall", bufs=2)
            for h in range(HG):
                if inter is not None:
                    inter(h)
                ps = psc.tile([C, 2 * (D + 1) + C], F32, tag="scanps")
                nu = ps[:, 0:D + 1]
                U = ps[:, D + 1:2 * (D + 1)]
                A_ps = ps[:, 2 * (D + 1):2 * (D + 1) + C]
                vh = v_sb[:, h * (D + 1):(h + 1) * (D + 1)]
                qT = qT3[:, h, :]
                kT = kT3[:, h, :]

                nc.tensor.matmul(A_ps, kT, qT, start=True, stop=True)
                A_sb = sc.tile([C, C], BF16, tag="A")
                nc.vector.scalar_tensor_tensor(
                    out=A_sb[:], in0=A_ps,
                    scalar=ainv_t[:, c * HG + h:c * HG + h + 1], in1=mask_t[:],
                    op0=Alu.mult, op1=Alu.mult)

                nc.tensor.matmul(U, khat[:, bass.ts(h, D)], vh,
                                 start=True, stop=True)
                nc.tensor.matmul(nu, A_sb[:], vh, start=True, stop=False)
                nc.tensor.matmul(nu, qT, S_cur[h][:], start=False, stop=True)

                S_new = spool.tile([C, D + 1], BF16, tag=f"s{h}")
                nc.vector.scalar_tensor_tensor(
                    out=S_new[:], in0=S_cur[h][:],
                    scalar=acb_t[:, c * HG + h:c * HG + h + 1],
                    in1=U, op0=Alu.mult, op1=Alu.add)
                S_cur[h] = S_new

                rd = sc.tile([C, 1], F32, tag="rd")
                nc.vector.tensor_scalar_add(rd[:], nu[:, D:D + 1], EPS)
                nc.vector.reciprocal(rd[:], rd[:])
                nc.vector.tensor_scalar_mul(y_all[:, bass.ts(h, D)],
                                            nu[:, 0:D], rd[:])

            if DBG and c == 0:
                nc.sync.dma_start(dbg_y, y_all[:])
            yT3 = ytb.tile([C, HG, C], BF16, tag="yT3")
            (ytp_eng or nc.sync).dma_start(yT3[:], y_all[:], transpose=True)
            stash[(c, "yT3")] = yT3

        def outproj_group(c, o, yT3, out_sb, split_store):
            out_ps = pou.tile([C, GW], F32, tag="po")
            for h in range(HG):
                nc.tensor.matmul(out_ps[:], yT3[:, h, :],
                                 wo_t[h][:, bass.ts(o, GW)],
                                 start=(h == 0), stop=(h == HG - 1))
            nc.scalar.copy(out_sb[:, bass.ts(o, GW)], out_ps[:])
            if split_store:
                nc.sync.dma_start(out_d[bass.ts(c, C), bass.ts(o, GW)],
                                  out_sb[:, bass.ts(o, GW)])

        def emit_outproj(c, split_store=False, group_cb=None):
            yT3 = stash.pop((c, "yT3"))
            out_sb = ob.tile([C, HID], BF16, tag="osb")
            NO = HID // GW
            if group_cb is not None:
                return yT3, out_sb
            for o in range(NO):
                outproj_group(c, o, yT3, out_sb, split_store)
            if not split_store:
                nc.sync.dma_start(out_d[bass.ts(c, C), :], out_sb[:])

        for c in range(nch):
            emit_proj(c, tp_eng=nc.scalar if c == nch - 1 else None)
            if c >= 1:
                emit_scan(c - 1)
            if c >= 2:
                emit_outproj(c - 2)
        if nch >= 2:
            yT3_p, osb_p = emit_outproj(nch - 2, split_store=True, group_cb=True)
            emit_scan(nch - 1,
                      inter=lambda h: outproj_group(nch - 2, h, yT3_p, osb_p,
                                                    True),
                      ytp_eng=None)
        else:
            emit_scan(nch - 1)
        emit_outproj(nch - 1, split_store=True)

    nc.compile()
    return nc


def _get_nc(cfg_key="default", **cfg):
    if cfg_key not in _CACHE:
        _CACHE[cfg_key] = _build(cfg)
    return _CACHE[cfg_key]


def _bf16(a):
    import ml_dtypes
    return np.ascontiguousarray(a).astype(ml_dtypes.bfloat16)


def _block_w(W, fw):
    # [HID, fw] -> [C, NK*fw] with block k = W[k*128:(k+1)*128, :]
    return np.ascontiguousarray(
        W.reshape(NK, C, fw).transpose(1, 0, 2).reshape(C, NK * fw))


def make_in_maps(x, Wq, Wk, Wv, Wg, bg, Wo, bo, wdt="bf16"):
    cosr, sinr = _rope_tables()
    maskT = np.triu(np.ones((C, C), np.float32))
    import ml_dtypes
    identb = np.eye(C, dtype=ml_dtypes.bfloat16)
    x = np.asarray(x, np.float32)
    Wq, Wk, Wv = np.asarray(Wq), np.asarray(Wk), np.asarray(Wv)
    Wg, bg, Wo = np.asarray(Wg), np.asarray(bg), np.asarray(Wo)
    wcast = _bf16 if wdt == "bf16" else (
        lambda a: np.ascontiguousarray(a).astype(np.float32))
    xcast = _bf16 if wdt == "bf16" else (
        lambda a: np.ascontiguousarray(a).astype(np.float32))

    # host beta pipeline: beta = clip(sigmoid(x@Wg+bg)); per-chunk cumprods
    g = x.reshape(B * L, HID) @ Wg + bg          # [B*L, H]
    beta = np.clip(1.0 / (1.0 + np.exp(-g)), BETA_MIN, BETA_MAX)
    beta = beta.reshape(B, L, H).astype(np.float32)
    a = np.cumprod(beta.reshape(B, NCHUNK, C, H), axis=2).astype(np.float32)
    aC = a[:, :, C - 1:C, :]                      # [B, NCHUNK, 1, H]
    ainv = (1.0 / a).astype(np.float32)
    acdiv = (aC / a).astype(np.float32)
    acb = np.broadcast_to(aC, a.shape).astype(np.float32)

    def bvec(m, b, hsl):  # [B,NCHUNK,C,H] -> [C, NCHUNK*HG]
        return np.ascontiguousarray(
            m[b][:, :, hsl].transpose(1, 0, 2).reshape(C, NCHUNK * HG)
        ).astype(np.float32)

    in_maps = []
    for core in range(NCORES):
        b, hg = divmod(core, 4)
        cs = slice(hg * GW, (hg + 1) * GW)
        hsl = slice(hg * HG, (hg + 1) * HG)
        xTb = xcast(
            x[b].reshape(NCHUNK, C, NK, C).transpose(0, 3, 2, 1)
            .reshape(NCHUNK, C, HID))
        in_maps.append({
            "xTb": xTb,
            "wq": wcast(_block_w(Wq[:, cs], GW)),
            "wk": wcast(_block_w(Wk[:, cs], GW)),
            "wv": wcast(_block_w(Wv[:, cs], GW)),
            "wo": _bf16(Wo[cs, :].reshape(HG, C, HID).transpose(1, 0, 2)
                        .reshape(C, HG * HID)),
            "cosr": cosr, "sinr": sinr, "maskT": maskT,
            "identb": identb,
            "ainv": bvec(ainv, b, hsl),
            "acdiv": bvec(acdiv, b, hsl),
            "acb": bvec(acb, b, hsl),
        })
    return in_maps


def kernel(x, Wq, Wk, Wv, Wg, bg, Wo, bo, _trace=False, cfg_key="default", **cfg):
    from concourse.bass_utils import run_bass_kernel_spmd
    nc = _get_nc(cfg_key=cfg_key, **cfg)
    in_maps = make_in_maps(x, Wq, Wk, Wv, Wg, bg, Wo, bo,
                           wdt=cfg.get("wmode", "bf16"))
    res = run_bass_kernel_spmd(nc, in_maps, core_ids=list(range(NCORES)),
                               trace=_trace)
    out = np.zeros((B, L, HID), np.float32)
    for core in range(NCORES):
        b = core // 4
        out[b] += np.asarray(res.results[core]["out"], np.float32)
    out += np.asarray(bo, np.float32)[None, None, :]
    kernel._last_results = res
    return out


# revision 11
# speedup vs baseline: 1.0883x; 1.0211x over previous
"""Trainium2 Bass kernel for nn_DeltaNet_22488448762128 (v3).

Full-input contract: kernel(**inputs) takes unsharded numpy inputs, returns
the full [B, L, HID] output. Core = (b, hg): batch b in {0,1}, head-group hg
in {0..3} of HG=4 heads. Host sums the 4 partial outputs per batch + bo.

Design (per core, chunk c of 16 x 128 tokens):
  - PROJ: q/k/v = xT_blk.T @ W ; lhsT = x-block (f32r, self-loading matmul),
    rhs = W (bf16 moving, 1 cyc/row). PSUM q,k,v banks; drains on ACT; rope +
    phi on DVE (phi out bf16); khat = phik * (aC/a_t) in one DVE op.
  - beta/cumprod vectors (a, 1/a, aC/a, aC) computed on HOST from x@Wg and
    fed as small fp32 inputs (the g-projection is negligible host work).
  - q/k head-transposes (d-major for the scan) via ONE batched DMA-xbar
    transpose each (idle DMA engines; out AP [d, h, t], head-padded tile).
  - SCAN per head: A = kT.T@qT (bf16); A_sb = A*ainv[s]*mask; nu = A_sb.T@
    [V|1] + qT.T@S ; U = khat.T@[V|1]; S = aC*S + U; y = nu[:,:D]/(nu[:,D]+eps).
    nu|U|A packed in ONE psum bank (x2 bufs).
  - y batched DMA-transpose -> yT; OUTPROJ (2 chunks behind): out += yT.T@Wo
    (bf16 moving), 4x N=512 groups -> one [C, HID] store per chunk.
  - software pipeline per iter: PROJ(c), SCAN(c-1), OUTPROJ(c-2) so the PE
    never waits on DVE/DMA products; 16 warmup matmuls ramp the PE p-state.
"""

import math
import numpy as np

B, L, HID = 2, 2048, 2048
H, D = 16, 128
HG = 4
C = 128
NCHUNK = L // C
NK = HID // C
EPS = 1e-6
BETA_MIN, BETA_MAX = 0.8, 0.9995
NCORES = 8
GW = HG * D          # 512
half = D // 2
CP = C + 8           # padded head stride for 3D dma-transpose outputs

_CACHE = {}


def _rope_tables():
    inv_freq = (1.0 / (10000.0 ** (np.arange(half, dtype=np.float32) /
                                   np.float32(half)))).astype(np.float32)
    t = np.arange(L, dtype=np.float32)
    freqs = t[:, None] * inv_freq[None, :]
    cos = np.cos(freqs).astype(np.float32)
    sin = np.sin(freqs).astype(np.float32)

    def rearr(m):  # [L, half] -> [C, NCHUNK*half], block c at cols c*half
        return np.ascontiguousarray(
            m.reshape(NCHUNK, C, half).transpose(1, 0, 2).reshape(C, NCHUNK * half))
    return rearr(cos), rearr(sin)


def _build(cfg):
    import concourse.bass as bass
    import concourse.bacc as bacc
    import concourse.tile as tile
    import concourse.mybir as mybir
    from contextlib import ExitStack

    dt = mybir.dt
    F32 = dt.float32
    BF16 = dt.bfloat16
    F32R = dt.float32r
    Alu = mybir.AluOpType
    Act = mybir.ActivationFunctionType

    nch = cfg.get("nchunk", NCHUNK)
    nwarm = cfg.get("warm", 12)
    WMODE = cfg.get("wmode", "bf16")
    WDT = BF16 if WMODE == "bf16" else F32R
    XDT = BF16 if WMODE == "bf16" else F32R

    nc = bacc.Bacc("TRN2", target_bir_lowering=False, debug=False,
                   enable_asserts=False, num_devices=NCORES)

    # ---- DRAM I/O ----
    xT_d = nc.dram_tensor("xTb", [NCHUNK, C, HID], XDT, kind="ExternalInput").ap()
    wq_d = nc.dram_tensor("wq", [C, NK * GW], WDT, kind="ExternalInput").ap()
    wk_d = nc.dram_tensor("wk", [C, NK * GW], WDT, kind="ExternalInput").ap()
    wv_d = nc.dram_tensor("wv", [C, NK * GW], WDT, kind="ExternalInput").ap()
    wo_d = nc.dram_tensor("wo", [C, HG * HID], BF16, kind="ExternalInput").ap()
    cos_d = nc.dram_tensor("cosr", [C, NCHUNK * half], F32, kind="ExternalInput").ap()
    sin_d = nc.dram_tensor("sinr", [C, NCHUNK * half], F32, kind="ExternalInput").ap()
    mask_d = nc.dram_tensor("maskT", [C, C], F32, kind="ExternalInput").ap()
    idb_d = nc.dram_tensor("identb", [C, C], BF16, kind="ExternalInput").ap()
    ainv_d = nc.dram_tensor("ainv", [C, NCHUNK * HG], F32, kind="ExternalInput").ap()
    acdiv_d = nc.dram_tensor("acdiv", [C, NCHUNK * HG], F32, kind="ExternalInput").ap()
    acb_d = nc.dram_tensor("acb", [C, NCHUNK * HG], F32, kind="ExternalInput").ap()
    out_d = nc.dram_tensor("out", [L, HID], BF16, kind="ExternalOutput").ap()
    DBG = cfg.get("dbg", False)
    if DBG:
        dbg_q = nc.dram_tensor("dbg_q", [C, GW], F32, kind="ExternalOutput").ap()
        dbg_phiq = nc.dram_tensor("dbg_phiq", [C, GW], BF16, kind="ExternalOutput").ap()
        dbg_qT = nc.dram_tensor("dbg_qT", [C, HG * C], BF16, kind="ExternalOutput").ap()
        dbg_v = nc.dram_tensor("dbg_v", [C, HG * (D + 1)], BF16, kind="ExternalOutput").ap()
        dbg_y = nc.dram_tensor("dbg_y", [C, GW], BF16, kind="ExternalOutput").ap()
        dbg_w = nc.dram_tensor("dbg_w", [C, GW], WDT, kind="ExternalOutput").ap()
        dbg_kh = nc.dram_tensor("dbg_kh", [C, GW], BF16, kind="ExternalOutput").ap()

    NWSUB = 4               # weight quarters per projection
    KSUB = NK // NWSUB

    with ExitStack() as ctx:
        tc = ctx.enter_context(tile.TileContext(nc))

        cpool = ctx.enter_context(tc.tile_pool(name="consts", bufs=1))
        cos_t = cpool.tile([C, NCHUNK * half], F32, tag="cos")
        sin_t = cpool.tile([C, NCHUNK * half], F32, tag="sin")
        mask_t = cpool.tile([C, C], F32, tag="mask")
        ainv_t = cpool.tile([C, NCHUNK * HG], F32, tag="ainv")
        acdiv_t = cpool.tile([C, NCHUNK * HG], F32, tag="acdiv")
        acb_t = cpool.tile([C, NCHUNK * HG], F32, tag="acb")
        wsc = cpool.tile([C, GW + C], BF16, tag="wsc")   # warmup scratch
        idb_t = cpool.tile([C, C], BF16, tag="idb")

        wpool = ctx.enter_context(tc.tile_pool(name="w", bufs=1))
        wq_t = [wpool.tile([C, KSUB * GW], WDT, tag=f"wq{j}", name=f"wq{j}")
                for j in range(NWSUB)]
        wk_t = [wpool.tile([C, KSUB * GW], WDT, tag=f"wk{j}", name=f"wk{j}")
                for j in range(NWSUB)]
        wv_t = [wpool.tile([C, KSUB * GW], WDT, tag=f"wv{j}", name=f"wv{j}")
                for j in range(NWSUB)]
        wo_t = [wpool.tile([C, HID], BF16, tag=f"wo{h}", name=f"wo{h}")
                for h in range(HG)]

        # chunk-local SBUF pools
        xp = ctx.enter_context(tc.tile_pool(name="xp", bufs=cfg.get("xp", 3)))
        dr = ctx.enter_context(tc.tile_pool(name="dr", bufs=cfg.get("dr", 2)))
        rp = ctx.enter_context(tc.tile_pool(name="rp", bufs=2))
        ph = ctx.enter_context(tc.tile_pool(name="ph", bufs=2))
        tp = ctx.enter_context(tc.tile_pool(name="tp", bufs=2))
        sc = ctx.enter_context(tc.tile_pool(name="sc", bufs=3))
        spool = ctx.enter_context(tc.tile_pool(name="spool", bufs=2))
        ytb = ctx.enter_context(tc.tile_pool(name="ytb", bufs=2))
        ob = ctx.enter_context(tc.tile_pool(name="ob", bufs=cfg.get("ob", 3)))

        # PSUM: 3 (qkv) + 2 (scanps) + 2 (out) + 1 (warm) = 8 banks
        pqkv = ctx.enter_context(tc.tile_pool(name="pqkv", bufs=1, space="PSUM"))
        psc = ctx.enter_context(tc.tile_pool(name="psc", bufs=2, space="PSUM"))
        pou = ctx.enter_context(tc.tile_pool(name="pou", bufs=2, space="PSUM"))
        pwm = ctx.enter_context(tc.tile_pool(name="pwm", bufs=1, space="PSUM"))

        # ---- warmup the PE clock while initial DMAs land ----
        nc.vector.memset(wsc[:], 0.0)
        warm_ps = pwm.tile([C, GW], F32, tag="warm")
        for i in range(nwarm):
            nc.tensor.matmul(warm_ps[:], wsc[:, GW:GW + C], wsc[:, 0:GW],
                             start=True, stop=True, skip_group_check=True)

        # ---- initial DMAs: xtb(0) on SP; consts + weights on GPSIMD ----
        xtb_tiles = {}
        xtb_tiles[0] = xp.tile([C, HID], XDT, tag="xtb", name="xtb0")
        nc.sync.dma_start(xtb_tiles[0][:], xT_d[0])
        nc.scalar.dma_start(cos_t[:], cos_d)
        nc.scalar.dma_start(sin_t[:], sin_d)
        for wd, wt in ((wq_d, wq_t), (wk_d, wk_t), (wv_d, wv_t)):
            for j in range(NWSUB):
                js = bass.ts(j, KSUB * GW)
                nc.gpsimd.dma_start(wt[j][:], wd[:, js])
        nc.scalar.dma_start(acdiv_t[:], acdiv_d)
        nc.scalar.dma_start(ainv_t[:], ainv_d)
        nc.scalar.dma_start(acb_t[:], acb_d)
        nc.scalar.dma_start(mask_t[:], mask_d)
        nc.scalar.dma_start(idb_t[:], idb_d)
        for h in range(HG):
            for u in range(2):
                nc.gpsimd.dma_start(
                    wo_t[h][:, bass.ts(u, HID // 2)],
                    wo_d[:, h * HID + u * (HID // 2):
                         h * HID + (u + 1) * (HID // 2)])



        # ---- persistent state ----
        S_cur = []
        for h in range(HG):
            s0 = spool.tile([C, D + 1], BF16, tag=f"s{h}", name=f"s0_{h}")
            nc.vector.memset(s0[:], 0.0)
            S_cur.append(s0)

        stash = {}

        def emit_proj(c, tp_eng=None):
            xtb = xtb_tiles.pop(c)


            q_ps = pqkv.tile([C, GW], F32, tag="pq")
            k_ps = pqkv.tile([C, GW], F32, tag="pk")
            v_ps = pqkv.tile([C, GW], F32, tag="pv")
            for dst, wt in ((q_ps, wq_t), (k_ps, wk_t), (v_ps, wv_t)):
                for k in range(NK):
                    j, jj = divmod(k, KSUB)
                    nc.tensor.matmul(dst[:], xtb[:, bass.ts(k, C)],
                                     wt[j][:, bass.ts(jj, GW)],
                                     start=(k == 0), stop=(k == NK - 1))

            # drains (ACT)
            q_sb = dr.tile([C, GW], F32, tag="q")
            k_sb = dr.tile([C, GW], F32, tag="k")
            nc.scalar.copy(q_sb[:], q_ps[:])
            nc.scalar.copy(k_sb[:], k_ps[:])
            # rope (DVE)
            def rope(src, dst):
                se = src[:].rearrange("p (h d) -> p h d", h=HG)[:, :, 0:half]
                so = src[:].rearrange("p (h d) -> p h d", h=HG)[:, :, half:D]
                de = dst[:].rearrange("p (h d) -> p h d", h=HG)[:, :, 0:half]
                do = dst[:].rearrange("p (h d) -> p h d", h=HG)[:, :, half:D]
                cc = bass.AP(tensor=cos_t[:].tensor,
                             offset=cos_t[:, bass.ts(c, half)].offset,
                             ap=[cos_t[:].ap[0], [0, HG], [1, half]])
                ss = bass.AP(tensor=sin_t[:].tensor,
                             offset=sin_t[:, bass.ts(c, half)].offset,
                             ap=[sin_t[:].ap[0], [0, HG], [1, half]])
                tmp = rp.tile([C, GW], F32, tag="rtmp")
                t1 = tmp[:].rearrange("p (h d) -> p h d", h=HG)[:, :, 0:half]
                t2 = tmp[:].rearrange("p (h d) -> p h d", h=HG)[:, :, half:D]
                nc.vector.tensor_tensor(out=t1, in0=se, in1=cc, op=Alu.mult)
                nc.vector.tensor_tensor(out=t2, in0=so, in1=ss, op=Alu.mult)
                nc.vector.tensor_tensor(out=de, in0=t1, in1=t2, op=Alu.subtract)
                nc.vector.tensor_tensor(out=t1, in0=se, in1=ss, op=Alu.mult)
                nc.vector.tensor_tensor(out=t2, in0=so, in1=cc, op=Alu.mult)
                nc.vector.tensor_tensor(out=do, in0=t1, in1=t2, op=Alu.add)

            qr = rp.tile([C, GW], F32, tag="qr")
            kr = rp.tile([C, GW], F32, tag="kr")
            rope(q_sb, qr)
            rope(k_sb, kr)

            # phi = exp(min(x,0)) + relu(x)  (gpsimd min, ACT exp, DVE fuse)
            def phi(src, nm):
                tm = rp.tile([C, GW], F32, tag="rtmp")
                nc.vector.tensor_scalar(out=tm[:], in0=src[:], scalar1=0.0,
                                        scalar2=None, op0=Alu.min)
                te = rp.tile([C, GW], F32, tag="te")
                nc.scalar.activation(te[:], tm[:], Act.Exp)
                p = ph.tile([C, GW], BF16, tag=nm)
                nc.vector.scalar_tensor_tensor(out=p[:], in0=src[:],
                                               scalar=0.0, in1=te[:],
                                               op0=Alu.max, op1=Alu.add)
                return p

            phiq = phi(qr, "phiq")
            phik = phi(kr, "phik")

            # khat for all 4 heads in one DVE op: phik * acdiv[t,h] (bcast on d)
            khat = ph.tile([C, GW], BF16, tag="khat")
            av = bass.AP(tensor=acdiv_t[:].tensor,
                         offset=acdiv_t[:, bass.ts(c, HG)].offset,
                         ap=[acdiv_t[:].ap[0], [1, HG], [0, D]])
            nc.vector.tensor_tensor(
                out=khat[:].rearrange("p (h d) -> p h d", h=HG),
                in0=phik[:].rearrange("p (h d) -> p h d", h=HG),
                in1=av, op=Alu.mult)

            # batched DMA xbar transposes: [t, (h d)] -> [d, h, t]
            # (contiguous out tile; HW-validated in micro1)
            qT3 = tp.tile([C, HG, C], BF16, tag="qT3")
            kT3 = tp.tile([C, HG, C], BF16, tag="kT3")
            (tp_eng or nc.sync).dma_start(qT3[:], phiq[:], transpose=True)
            (tp_eng or nc.sync).dma_start(kT3[:], phik[:], transpose=True)
            if c + 1 < nch:
                nxt = xp.tile([C, HID], XDT, tag="xtb", name=f"xtb{c + 1}")
                nc.sync.dma_start(nxt[:], xT_d[c + 1])
                xtb_tiles[c + 1] = nxt

            # v drain late (keeps exp/phi ahead of it on ACT; needed next iter)
            v_sb = ph.tile([C, HG * (D + 1)], BF16, tag="v")
            v_aug = v_sb[:].rearrange("p (h e) -> p h e", e=D + 1)
            nc.scalar.copy(v_aug[:, :, 0:D],
                           v_ps[:].rearrange("p (h e) -> p h e", e=D))
            nc.vector.memset(v_aug[:, :, D:D + 1], 1.0)

            if DBG and c == 0:
                nc.sync.dma_start(dbg_q, q_sb[:])
                nc.sync.dma_start(dbg_phiq, phiq[:])
                nc.sync.dma_start(dbg_qT, qT3[:].rearrange("p h t -> p (h t)"))
                nc.sync.dma_start(dbg_v, v_sb[:])
                nc.sync.dma_start(dbg_w, wq_t[0][:, 0:GW])
                nc.sync.dma_start(dbg_kh, khat[:])
            stash[c] = dict(qT3=qT3, kT3=kT3, khat=khat, v_sb=v_sb)

        def emit_scan(c, inter=None, ytp_eng=None):
            st = stash.pop(c)
            qT3, kT3, khat, v_sb = st["qT3"], st["kT3"], st["khat"], st["v_sb"]
            y_all = sc.tile([C, GW], BF16, tag="yall", bufs=2)
            for h in range(HG):
                if inter is not None:
                    inter(h)
                ps = psc.tile([C, 2 * (D + 1) + C], F32, tag="scanps")
                nu = ps[:, 0:D + 1]
                U = ps[:, D + 1:2 * (D + 1)]
                A_ps = ps[:, 2 * (D + 1):2 * (D + 1) + C]
                vh = v_sb[:, h * (D + 1):(h + 1) * (D + 1)]
                qT = qT3[:, h, :]
                kT = kT3[:, h, :]

                nc.tensor.matmul(A_ps, kT, qT, start=True, stop=True)
                A_sb = sc.tile([C, C], BF16, tag="A")
                nc.vector.scalar_tensor_tensor(
                    out=A_sb[:], in0=A_ps,
                    scalar=ainv_t[:, c * HG + h:c * HG + h + 1], in1=mask_t[:],
                    op0=Alu.mult, op1=Alu.mult)

                nc.tensor.matmul(U, khat[:, bass.ts(h, D)], vh,
                                 start=True, stop=True)
                nc.tensor.matmul(nu, A_sb[:], vh, start=True, stop=False)
                nc.tensor.matmul(nu, qT, S_cur[h][:], start=False, stop=True)

                S_new = spool.tile([C, D + 1], BF16, tag=f"s{h}")
                nc.vector.scalar_tensor_tensor(
                    out=S_new[:], in0=S_cur[h][:],
                    scalar=acb_t[:, c * HG + h:c * HG + h + 1],
                    in1=U, op0=Alu.mult, op1=Alu.add)
                S_cur[h] = S_new

                rd = sc.tile([C, 1], F32, tag="rd")
                nc.vector.tensor_scalar_add(rd[:], nu[:, D:D + 1], EPS)
                nc.vector.reciprocal(rd[:], rd[:])
                nc.vector.tensor_scalar_mul(y_all[:, bass.ts(h, D)],
                                            nu[:, 0:D], rd[:])

            if DBG and c == 0:
                nc.sync.dma_start(dbg_y, y_all[:])
            yT3 = ytb.tile([C, HG, C], BF16, tag="yT3")
            (ytp_eng or nc.sync).dma_start(yT3[:], y_all[:], transpose=True)
            stash[(c, "yT3")] = yT3

        def outproj_group(c, o, yT3, out_sb, split_store):
            out_ps = pou.tile([C, GW], F32, tag="po")
            for h in range(HG):
                nc.tensor.matmul(out_ps[:], yT3[:, h, :],
                                 wo_t[h][:, bass.ts(o, GW)],
                                 start=(h == 0), stop=(h == HG - 1))
            nc.scalar.copy(out_sb[:, bass.ts(o, GW)], out_ps[:])
            if split_store:
                nc.sync.dma_start(out_d[bass.ts(c, C), bass.ts(o, GW)],
                                  out_sb[:, bass.ts(o, GW)])

        def emit_outproj(c, split_store=False, group_cb=None):
            yT3 = stash.pop((c, "yT3"))
            out_sb = ob.tile([C, HID], BF16, tag="osb")
            NO = HID // GW
            if group_cb is not None:
                return yT3, out_sb
            for o in range(NO):
                outproj_group(c, o, yT3, out_sb, split_store)
            if not split_store:
                nc.sync.dma_start(out_d[bass.ts(c, C), :], out_sb[:])

        for c in range(nch):
            emit_proj(c, tp_eng=nc.scalar if c == nch - 1 else None)
            if c >= 1:
                emit_scan(c - 1)
            if c >= 2:
                emit_outproj(c - 2)
        if nch >= 2:
            yT3_p, osb_p = emit_outproj(nch - 2, split_store=True, group_cb=True)
            emit_scan(nch - 1,
                      inter=lambda h: outproj_group(nch - 2, h, yT3_p, osb_p,
                                                    True),
                      ytp_eng=None)
        else:
            emit_scan(nch - 1)
        emit_outproj(nch - 1, split_store=True)

    nc.compile()
    return nc


def _get_nc(cfg_key="default", **cfg):
    if cfg_key not in _CACHE:
        _CACHE[cfg_key] = _build(cfg)
    return _CACHE[cfg_key]


def _bf16(a):
    import ml_dtypes
    return np.ascontiguousarray(a).astype(ml_dtypes.bfloat16)


def _block_w(W, fw):
    # [HID, fw] -> [C, NK*fw] with block k = W[k*128:(k+1)*128, :]
    return np.ascontiguousarray(
        W.reshape(NK, C, fw).transpose(1, 0, 2).reshape(C, NK * fw))


def make_in_maps(x, Wq, Wk, Wv, Wg, bg, Wo, bo, wdt="bf16"):
    cosr, sinr = _rope_tables()
    maskT = np.triu(np.ones((C, C), np.float32))
    import ml_dtypes
    identb = np.eye(C, dtype=ml_dtypes.bfloat16)
    x = np.asarray(x, np.float32)
    Wq, Wk, Wv = np.asarray(Wq), np.asarray(Wk), np.asarray(Wv)
    Wg, bg, Wo = np.asarray(Wg), np.asarray(bg), np.asarray(Wo)
    wcast = _bf16 if wdt == "bf16" else (
        lambda a: np.ascontiguousarray(a).astype(np.float32))
    xcast = _bf16 if wdt == "bf16" else (
        lambda a: np.ascontiguousarray(a).astype(np.float32))

    # host beta pipeline: beta = clip(sigmoid(x@Wg+bg)); per-chunk cumprods
    g = x.reshape(B * L, HID) @ Wg + bg          # [B*L, H]
    beta = np.clip(1.0 / (1.0 + np.exp(-g)), BETA_MIN, BETA_MAX)
    beta = beta.reshape(B, L, H).astype(np.float32)
    a = np.cumprod(beta.reshape(B, NCHUNK, C, H), axis=2).astype(np.float32)
    aC = a[:, :, C - 1:C, :]                      # [B, NCHUNK, 1, H]
    ainv = (1.0 / a).astype(np.float32)
    acdiv = (aC / a).astype(np.float32)
    acb = np.broadcast_to(aC, a.shape).astype(np.float32)

    def bvec(m, b, hsl):  # [B,NCHUNK,C,H] -> [C, NCHUNK*HG]
        return np.ascontiguousarray(
            m[b][:, :, hsl].transpose(1, 0, 2).reshape(C, NCHUNK * HG)
        ).astype(np.float32)

    in_maps = []
    for core in range(NCORES):
        b, hg = divmod(core, 4)
        cs = slice(hg * GW, (hg + 1) * GW)
        hsl = slice(hg * HG, (hg + 1) * HG)
        xTb = xcast(
            x[b].reshape(NCHUNK, C, NK, C).transpose(0, 3, 2, 1)
            .reshape(NCHUNK, C, HID))
        in_maps.append({
            "xTb": xTb,
            "wq": wcast(_block_w(Wq[:, cs], GW)),
            "wk": wcast(_block_w(Wk[:, cs], GW)),
            "wv": wcast(_block_w(Wv[:, cs], GW)),
            "wo": _bf16(Wo[cs, :].reshape(HG, C, HID).transpose(1, 0, 2)
                        .reshape(C, HG * HID)),
            "cosr": cosr, "sinr": sinr, "maskT": maskT,
            "identb": identb,
            "ainv": bvec(ainv, b, hsl),
            "acdiv": bvec(acdiv, b, hsl),
            "acb": bvec(acb, b, hsl),
        })
    return in_maps


def kernel(x, Wq, Wk, Wv, Wg, bg, Wo, bo, _trace=False, cfg_key="default", **cfg):
    from concourse.bass_utils import run_bass_kernel_spmd
    nc = _get_nc(cfg_key=cfg_key, **cfg)
    in_maps = make_in_maps(x, Wq, Wk, Wv, Wg, bg, Wo, bo,
                           wdt=cfg.get("wmode", "bf16"))
    res = run_bass_kernel_spmd(nc, in_maps, core_ids=list(range(NCORES)),
                               trace=_trace)
    out = np.zeros((B, L, HID), np.float32)
    for core in range(NCORES):
        b = core // 4
        out[b] += np.asarray(res.results[core]["out"], np.float32)
    out += np.asarray(bo, np.float32)[None, None, :]
    kernel._last_results = res
    return out
